# revision 1
# baseline (speedup 1.0000x reference)
"""Trainium2 Bass kernel for a dense transformer block (B=4, T=2048, C=1024, H=16).

Sharding: zero-collective. Each of the 8 cores owns (batch b, query-half h):
  core c -> b = c//2, half = c%2, query tokens = xb[half*1024 : half*1024+1024].
Per core (uniform SPMD program; all per-core variation is in the input data):
  - LN1 stats for the whole batch (bn_stats on x natural).
  - qkv computed from transposed x (xT) with LN folded in post-matmul:
      ln1(x) @ W = r .* (x @ (g.*W)) + (-mu*r) .* colsum(g.*W) + (b@ (g.*W) + b_attn)
    q/k kept transposed [d, t] for QK^T; v computed in natural [t, d] layout,
    augmented with a ones column so S@V also produces the softmax denominator.
  - full-rectangle attention (16 k-tiles per 512-query chunk) with host-supplied
    0/1 causal masks (per-core data, so the instruction stream is identical on
    all cores). P = exp(S/8) in bf16.
  - proj (local, full head dim), residual, LN2, fc+gelu, fc2, residual.
All matmuls run in bf16 with f32 PSUM accumulation; the residual stream,
softmax denominators and layernorm statistics stay f32.
"""
import sys
import math
import contextlib

for _p in ("/opt/trn_rl_repo", "/root/.axon_site/_ro/trn_rl_repo"):
    if _p not in sys.path:
        sys.path.append(_p)

import numpy as np
import ml_dtypes

import concourse.bass as bass
import concourse.bacc as bacc
import concourse.mybir as mybir
import concourse.tile as tile
from concourse.bass_utils import run_bass_kernel_spmd

F32 = mybir.dt.float32
BF16 = mybir.dt.bfloat16
AF = mybir.ActivationFunctionType
OP = mybir.AluOpType
BF = ml_dtypes.bfloat16

B, T, C, H = 4, 2048, 1024, 16
HD = C // H              # 64
EPS = 1e-5
TB = T                   # tokens per batch (2048)
TQ = T // 2              # query tokens per core (1024)
CK = C // 128            # 8 contraction k-tiles over C
NTT = TB // 128          # 16 token tiles per batch
NQT = TQ // 128          # 8 token tiles per core's queries
G = 4                    # head groups
HG = H // G              # 4 heads per group
DG = HG * HD             # 256 cols per group (per q/k/v)
FC = 4 * C               # 4096
NGT = FC // 128          # 32 fc tiles


def build_nc():
    nc = bacc.Bacc("TRN2", target_bir_lowering=False, debug=False, num_devices=8)

    dt_in = {
        # bf16 matmul operands
        "xT": ([C, TB], BF16), "qxT": ([C, TQ], BF16),
        "wqkv": ([C, 3 * C], BF16), "b1col": ([C, 1], BF16),
        "wproj": ([C, C], BF16), "wfc": ([C, FC], BF16),
        "wfc2": ([FC, C], BF16), "masks": ([16, 128, 512], BF16),
        # f32 data
        "xn": ([TB, C], F32), "xr": ([TQ, C], F32),
        "bq_col": ([3 * C, 1], F32), "bq_row": ([1, 3 * C], F32),
        "g1col": ([C, 1], F32),
        "bproj_bc": ([128, C], F32), "g2bc": ([128, C], F32),
        "b2bc": ([128, C], F32), "bfc_col": ([FC, 1], F32),
        "bfc2_bc": ([128, C], F32),
    }
    d = {k: nc.dram_tensor(k, sh, dt, kind="ExternalInput").ap()
         for k, (sh, dt) in dt_in.items()}
    out = nc.dram_tensor("out", [TQ, C], F32, kind="ExternalOutput").ap()

    with tile.TileContext(nc) as tc:
        with contextlib.ExitStack() as ctx:
            _build_body(nc, tc, ctx, d, out)
    nc.compile()
    return nc


def _build_body(nc, tc, ctx, d, out):
    pool = lambda name, bufs, **kw: ctx.enter_context(
        tc.tile_pool(name=name, bufs=bufs, **kw))

    cons = pool("cons", 1)
    small = pool("small", 2)
    stats = pool("stats", 3)
    ps = pool("ps", 4, space="PSUM")
    dram = pool("dram", 2, space="DRAM")

    # ---- constants ----
    ones_col = cons.tile([128, 1], BF16)
    nc.vector.memset(ones_col, 1.0)
    ones_t = cons.tile([128, 64], BF16)
    nc.vector.memset(ones_t, 1.0)
    eps_t = cons.tile([128, 1], F32)
    nc.vector.memset(eps_t, EPS)
    ident = cons.tile([128, 128], BF16)
    from concourse.masks import make_identity
    make_identity(nc, ident)

    g1sb = cons.tile([128, CK], F32)
    nc.sync.dma_start(out=g1sb, in_=d["g1col"].rearrange("(k p) o -> p (k o)", p=128))
    b1sb = cons.tile([128, CK], BF16)
    nc.sync.dma_start(out=b1sb, in_=d["b1col"].rearrange("(k p) o -> p (k o)", p=128))

    r_col = cons.tile([128, NTT], F32)
    mr_col = cons.tile([128, NTT], F32)
    rq_col = cons.tile([128, NQT], F32)
    mrq_col = cons.tile([128, NQT], F32)

    # ---- phase 0: LN1 stats over the full batch (and over own queries) ----
    def ln_stats(src_ap, n_tiles, rc, mrc):
        for tt in range(n_tiles):
            xt_f = stats.tile([128, C], F32, name="xt_f")
            nc.sync.dma_start(out=xt_f, in_=src_ap[tt * 128:(tt + 1) * 128, :])
            st = stats.tile([128, 2, 6], F32, name="st")
            resh = xt_f.rearrange("p (n f) -> p n f", f=512)
            for i in range(2):
                nc.vector.bn_stats(out=st[:, i, :], in_=resh[:, i, :])
            mv = stats.tile([128, 2], F32, name="mv")
            nc.vector.bn_aggr(out=mv, in_=st)
            sd = stats.tile([128, 1], F32, name="sd")
            nc.scalar.activation(sd, mv[:, 1:2], AF.Sqrt, bias=eps_t)
            nc.vector.reciprocal(rc[:, tt:tt + 1], sd)
            nc.vector.tensor_tensor(mrc[:, tt:tt + 1], mv[:, 0:1],
                                    rc[:, tt:tt + 1], op=OP.mult)
            nc.scalar.mul(mrc[:, tt:tt + 1], mrc[:, tt:tt + 1], -1.0)

    ln_stats(d["xn"], NTT, r_col, mr_col)
    ln_stats(d["xr"], NQT, rq_col, mrq_col)

    ypool = pool("ypool", 1)
    yT2 = ypool.tile([128, H // 2, TQ], BF16)

    # broadcast stats rows across partitions via a DRAM roundtrip
    attn_ctx0 = contextlib.ExitStack()
    abuf = attn_ctx0.enter_context(tc.tile_pool(name="abuf", bufs=1))
    r_bc = abuf.tile([128, TB], F32)
    mr_bc = abuf.tile([128, TB], F32)
    rq_bc = abuf.tile([128, TQ], F32)
    mrq_bc = abuf.tile([128, TQ], F32)

    def row_bcast(col_tile, n_tiles, dst):
        scr = dram.tile([n_tiles, 128], F32, name="scr")
        nc.gpsimd.dma_start(out=scr.rearrange("t p -> p t"),
                            in_=col_tile[:, 0:n_tiles])
        flat = scr.rearrange("t p -> (t p)").unsqueeze(0)
        nc.gpsimd.dma_start(out=dst, in_=flat.to_broadcast([128, n_tiles * 128]))

    row_bcast(r_col, NTT, r_bc)
    row_bcast(mr_col, NTT, mr_bc)
    row_bcast(rq_col, NQT, rq_bc)
    row_bcast(mrq_col, NQT, mrq_bc)

    # ---- attention-scope pools ----
    masksb = abuf.tile([128, 16, 512], BF16)
    nc.sync.dma_start(out=masksb, in_=d["masks"].rearrange("k p q -> p k q"))
    ones_mask = abuf.tile([128, 512], BF16)
    nc.vector.memset(ones_mask, 1.0)

    attn_ctx = contextlib.ExitStack()
    apool = lambda name, bufs, **kw: attn_ctx.enter_context(
        tc.tile_pool(name=name, bufs=bufs, **kw))
    wp = apool("wp", 1)
    xtp = apool("xtp", 2)
    qkv = apool("qkv", 1)
    pp = apool("pp", 14)
    stg = apool("stg", 2)
    qke = apool("qke", 2)
    psy = apool("psy", 3, space="PSUM")
    psb = apool("psb", 1, space="PSUM")

    qT = qkv.tile([128, HG, TQ], BF16, name="qT")
    kT = qkv.tile([128, HG, TB], BF16, name="kT")
    va = qkv.tile([128, NTT, HG, 128], BF16, name="va")
    nc.vector.memset(qT, 0.0)
    nc.vector.memset(kT, 0.0)
    nc.vector.memset(va, 0.0)
    nc.vector.memset(va[:, :, :, HD:HD + 1], 1.0)

    for g in range(G):
        # -- weights for this head group, scaled by ln1 gain --
        wt = wp.tile([128, CK, 3 * DG], BF16, name="wt")
        for kt in range(CK):
            for j, base in enumerate((0, C, 2 * C)):
                nc.sync.dma_start(
                    out=wt[:, kt, j * DG:(j + 1) * DG],
                    in_=d["wqkv"][kt * 128:(kt + 1) * 128,
                                  base + g * DG: base + (g + 1) * DG])
            nc.vector.tensor_scalar_mul(wt[:, kt, :], in0=wt[:, kt, :],
                                        scalar1=g1sb[:, kt:kt + 1])

        # -- s (colsum) and beta (b1 @ W' + b_attn) for q,k in column form --
        s_col, b_col = {}, {}
        for xi, xb in ((0, 0), (1, DG)):           # 0=q, 1=k
            for dt_ in range(2):
                ps_s = ps.tile([128, 512], F32, name="ps")
                ps_b = ps.tile([128, 512], F32, name="ps")
                for kt in range(CK):
                    w_sl = wt[:, kt, xb + dt_ * 128: xb + (dt_ + 1) * 128]
                    nc.tensor.matmul(ps_s[:, 0:1], w_sl, ones_col,
                                     start=(kt == 0), stop=(kt == CK - 1))
                    nc.tensor.matmul(ps_b[:, 0:1], w_sl, b1sb[:, kt:kt + 1],
                                     start=(kt == 0), stop=(kt == CK - 1))
                sc = small.tile([128, 1], F32, name=f"sc{xi}{dt_}")
                nc.scalar.copy(sc, ps_s[:, 0:1])
                s_col[(xi, dt_)] = sc
                bq_sl = small.tile([128, 1], F32, name=f"bq{xi}{dt_}")
                nc.sync.dma_start(
                    out=bq_sl,
                    in_=d["bq_col"][xi * C + g * DG + dt_ * 128:
                                    xi * C + g * DG + (dt_ + 1) * 128, :])
                bc = small.tile([128, 1], F32, name=f"bc{xi}{dt_}")
                nc.vector.tensor_tensor(bc, ps_b[:, 0:1], bq_sl, op=OP.add)
                b_col[(xi, dt_)] = bc

        # -- s and beta for v in row form, broadcast via DRAM roundtrip --
        ps_sv = ps.tile([128, 512], F32, name="ps")
        ps_bv = ps.tile([128, 512], F32, name="ps")
        for kt in range(CK):
            wv = wt[:, kt, 2 * DG:3 * DG]
            nc.tensor.matmul(ps_sv[0:1, 0:DG], ones_col, wv,
                             start=(kt == 0), stop=(kt == CK - 1))
            nc.tensor.matmul(ps_bv[0:1, 0:DG], b1sb[:, kt:kt + 1], wv,
                             start=(kt == 0), stop=(kt == CK - 1))
        sv_row = small.tile([1, DG], F32, name="sv_row")
        nc.scalar.copy(sv_row, ps_sv[0:1, 0:DG])
        bqv_sl = small.tile([1, DG], F32, name="bqv_sl")
        nc.sync.dma_start(out=bqv_sl,
                          in_=d["bq_row"][0:1, 2 * C + g * DG: 2 * C + (g + 1) * DG])
        bv_row = small.tile([1, DG], F32, name="bv_row")
        nc.vector.tensor_tensor(bv_row, ps_bv[0:1, 0:DG], bqv_sl, op=OP.add)
        sv_bc = small.tile([128, DG], F32, name="sv_bc")
        bv_bc = small.tile([128, DG], F32, name="bv_bc")
        for src, dst in ((sv_row, sv_bc), (bv_row, bv_bc)):
            scr2 = dram.tile([1, DG], F32, name="scr2")
            nc.gpsimd.dma_start(out=scr2, in_=src)
            nc.gpsimd.dma_start(out=dst, in_=scr2.to_broadcast([128, DG]))

        # -- qkv matmuls --
        def qk_evict(psrc, dst, dt_, sl, rbc_sl, mrbc_sl, sc, bc):
            e1 = qke.tile([128, 512], F32, name="e1")
            nc.vector.tensor_tensor(e1, psrc, rbc_sl, op=OP.mult)
            nc.vector.scalar_tensor_tensor(e1, in0=mrbc_sl, scalar=sc, in1=e1,
                                           op0=OP.mult, op1=OP.add)
            nc.vector.tensor_scalar_add(dst[0:64, 2 * dt_, sl], in0=e1[0:64, :],
                                        scalar1=bc[0:64, :])
            nc.vector.tensor_scalar_add(dst[64:128, 2 * dt_ + 1, sl],
                                        in0=e1[64:128, :], scalar1=bc[64:128, :])

        for ch in range(4):                      # k/v over the full batch
            xt = xtp.tile([128, CK, 512], BF16, name="xt")
            nc.sync.dma_start(
                out=xt,
                in_=d["xT"].rearrange("(k p) t -> p k t", p=128)[:, :, ch * 512:(ch + 1) * 512])
            for dt_ in range(2):                 # k
                psk = ps.tile([128, 512], F32, name="ps")
                for kt in range(CK):
                    nc.tensor.matmul(psk, wt[:, kt, DG + dt_ * 128:DG + (dt_ + 1) * 128],
                                     xt[:, kt, :], start=(kt == 0), stop=(kt == CK - 1))
                qk_evict(psk, kT, dt_, slice(ch * 512, (ch + 1) * 512),
                         r_bc[:, ch * 512:(ch + 1) * 512],
                         mr_bc[:, ch * 512:(ch + 1) * 512],
                         s_col[(1, dt_)], b_col[(1, dt_)])
            for tl in range(4):                  # v (natural layout)
                tt = ch * 4 + tl
                psv = ps.tile([128, 512], F32, name="ps")
                for kt in range(CK):
                    nc.tensor.matmul(psv[:, 0:DG], xt[:, kt, tl * 128:(tl + 1) * 128],
                                     wt[:, kt, 2 * DG:3 * DG],
                                     start=(kt == 0), stop=(kt == CK - 1))
                zt = qke.tile([128, DG], F32, name="zt")
                nc.vector.scalar_tensor_tensor(zt, in0=sv_bc,
                                               scalar=mr_col[:, tt:tt + 1],
                                               in1=bv_bc, op0=OP.mult, op1=OP.add)
                nc.vector.scalar_tensor_tensor(
                    va[:, tt, :, 0:HD],
                    in0=psv[:, 0:DG].rearrange("p (h d) -> p h d", h=HG),
                    scalar=r_col[:, tt:tt + 1],
                    in1=zt.rearrange("p (h d) -> p h d", h=HG),
                    op0=OP.mult, op1=OP.add)
        for ch in range(2):                      # q over own queries
            qxt = xtp.tile([128, CK, 512], BF16, name="qxt")
            nc.sync.dma_start(
                out=qxt,
                in_=d["qxT"].rearrange("(k p) t -> p k t", p=128)[:, :, ch * 512:(ch + 1) * 512])
            for dt_ in range(2):
                psq = ps.tile([128, 512], F32, name="ps")
                for kt in range(CK):
                    nc.tensor.matmul(psq, wt[:, kt, dt_ * 128:(dt_ + 1) * 128],
                                     qxt[:, kt, :], start=(kt == 0), stop=(kt == CK - 1))
                qk_evict(psq, qT, dt_, slice(ch * 512, (ch + 1) * 512),
                         rq_bc[:, ch * 512:(ch + 1) * 512],
                         mrq_bc[:, ch * 512:(ch + 1) * 512],
                         s_col[(0, dt_)], b_col[(0, dt_)])

        # -- attention for the 4 heads of this group --
        for hg in range(HG):
            h = g * HG + hg
            rb = (hg % 2) * 64
            for slot in range(2):
                psy_t = psy.tile([128, 512], F32, name="py")
                P_list = []
                for kt in range(16):
                    pss = ps.tile([128, 512], F32, name="ps")
                    nc.tensor.matmul(pss,
                                     kT[:, hg, kt * 128:(kt + 1) * 128],
                                     qT[:, hg, slot * 512:(slot + 1) * 512],
                                     start=True, stop=True)
                    P_t = pp.tile([128, 512], BF16, name="P")
                    nc.scalar.activation(P_t, pss, AF.Exp, scale=1.0 / math.sqrt(HD))
                    msk_sl = (masksb[:, kt, :] if slot == 0
                              else None if kt < 4
                              else masksb[:, kt - 4, :])
                    if msk_sl is not None:
                        nc.vector.tensor_mul(P_t, P_t, msk_sl)
                    P_list.append(P_t)
                for kt in range(16):
                    nc.tensor.matmul(psy_t, va[:, kt, hg, :], P_list[kt],
                                     start=(kt == 0), stop=(kt == 15))
                rec = stg.tile([128, 512], F32, name="rec")
                nc.vector.reciprocal(rec[64:65, :], psy_t[64:65, :])
                recb = stg.tile([128, 512], BF16, name="recb")
                nc.vector.tensor_copy(recb[64:65, :], rec[64:65, :])
                pbc = psb.tile([64, 512], F32, name="pb")
                nc.tensor.matmul(pbc, ones_t[64:65, :], recb[64:65, :],
                                 start=True, stop=True)
                yf = stg.tile([64, 512], F32, name="yf")
                nc.scalar.copy(yf, psy_t[0:64, :])
                yst = stg.tile([64, 512], BF16, name="yst")
                nc.vector.tensor_tensor(yst, yf, pbc, op=OP.mult)
                nc.sync.dma_start(
                    out=yT2[rb:rb + 64, h // 2, slot * 512:(slot + 1) * 512],
                    in_=yst)

    attn_ctx.close()
    attn_ctx0.close()

    # ---- proj + residual ----
    mlp = pool("mlp", 1)
    mstr = pool("mstr", 2)
    wstream = pool("wstream", 3)
    c2 = pool("c2", 1)
    x2 = mlp.tile([128, NQT, C], F32)
    bproj_sb = c2.tile([128, C], F32)
    nc.sync.dma_start(out=bproj_sb, in_=d["bproj_bc"])
    g2sb = c2.tile([128, C], F32)
    nc.sync.dma_start(out=g2sb, in_=d["g2bc"])
    b2sb = c2.tile([128, C], F32)
    nc.sync.dma_start(out=b2sb, in_=d["b2bc"])
    bfc2_sb = c2.tile([128, C], F32)
    nc.sync.dma_start(out=bfc2_sb, in_=d["bfc2_bc"])
    bfc_sb = c2.tile([128, NGT], F32)
    nc.sync.dma_start(out=bfc_sb,
                      in_=d["bfc_col"].rearrange("(k p) o -> p (k o)", p=128))

    wpj = []
    for kt in range(CK):
        w = c2.tile([128, C], BF16, name=f"wpj{kt}")
        nc.sync.dma_start(out=w, in_=d["wproj"][kt * 128:(kt + 1) * 128, :])
        wpj.append(w)

    for m in range(NQT):
        xr_t = mstr.tile([128, C], F32, name="xr_t")
        nc.sync.dma_start(out=xr_t, in_=d["xr"][m * 128:(m + 1) * 128, :])
        for n in range(2):
            psp = ps.tile([128, 512], F32, name="ps")
            for kt in range(CK):
                nc.tensor.matmul(psp, yT2[:, kt, m * 128:(m + 1) * 128],
                                 wpj[kt][:, n * 512:(n + 1) * 512],
                                 start=(kt == 0), stop=(kt == CK - 1))
            sl = slice(n * 512, (n + 1) * 512)
            nc.vector.tensor_tensor(x2[:, m, sl], psp, xr_t[:, sl], op=OP.add)
            nc.vector.tensor_tensor(x2[:, m, sl], x2[:, m, sl], bproj_sb[:, sl],
                                    op=OP.add)

    # ---- LN2 + transpose ----
    hT = mlp.tile([128, CK, TQ], BF16)
    for m in range(NQT):
        st = stats.tile([128, 2, 6], F32, name="st")
        resh = x2[:, m, :].rearrange("p (n f) -> p n f", f=512)
        for i in range(2):
            nc.vector.bn_stats(out=st[:, i, :], in_=resh[:, i, :])
        mv = stats.tile([128, 2], F32, name="mv")
        nc.vector.bn_aggr(out=mv, in_=st)
        sd = stats.tile([128, 1], F32, name="sd")
        nc.scalar.activation(sd, mv[:, 1:2], AF.Sqrt, bias=eps_t)
        r2 = stats.tile([128, 1], F32, name="r2")
        nc.vector.reciprocal(r2, sd)
        hm = mstr.tile([128, C], F32, name="hm")
        nc.vector.tensor_scalar(hm, in0=x2[:, m, :], scalar1=mv[:, 0:1],
                                scalar2=r2, op0=OP.subtract, op1=OP.mult)
        nc.vector.tensor_tensor(hm, hm, g2sb, op=OP.mult)
        hmb = mstr.tile([128, C], BF16, name="hmb")
        nc.vector.tensor_tensor(hmb, hm, b2sb, op=OP.add)
        for ck in range(CK):
            pst = ps.tile([128, 512], F32, name="ps")
            pstv = pst.bitcast(BF16)[:, 0:128]
            nc.tensor.transpose(pstv, hmb[:, ck * 128:(ck + 1) * 128], ident)
            nc.scalar.copy(hT[:, ck, m * 128:(m + 1) * 128], pstv)

    # ---- MLP ----
    hid = mlp.tile([128, NGT, 512], BF16, name="hid")
    psacc = pool("psacc", 4, space="PSUM")
    ostg = pool("ostg", 3)
    for th in range(2):
        for gtg in range(NGT // 4):
            wfcg = wstream.tile([128, CK, 512], BF16, name="wfcg")
            for kt in range(CK):
                nc.sync.dma_start(out=wfcg[:, kt, :],
                                  in_=d["wfc"][kt * 128:(kt + 1) * 128,
                                               gtg * 512:(gtg + 1) * 512])
            for gi in range(4):
                gt = gtg * 4 + gi
                psf = ps.tile([128, 512], F32, name="ps")
                for kt in range(CK):
                    nc.tensor.matmul(psf, wfcg[:, kt, gi * 128:(gi + 1) * 128],
                                     hT[:, kt, th * 512:(th + 1) * 512],
                                     start=(kt == 0), stop=(kt == CK - 1))
                nc.scalar.activation(hid[:, gt, :], psf, AF.Gelu,
                                     bias=bfc_sb[:, gt:gt + 1])
        for n in range(2):
            accs = [psacc.tile([128, 512], F32, name="acc") for _ in range(4)]
            for gkt in range(NGT):
                wf2 = wstream.tile([128, 512], BF16, name="wf2")
                nc.sync.dma_start(out=wf2,
                                  in_=d["wfc2"][gkt * 128:(gkt + 1) * 128,
                                                n * 512:(n + 1) * 512])
                for ml_ in range(4):
                    nc.tensor.matmul(accs[ml_],
                                     hid[:, gkt, ml_ * 128:(ml_ + 1) * 128],
                                     wf2, start=(gkt == 0), stop=(gkt == NGT - 1))
            for ml_ in range(4):
                m = th * 4 + ml_
                osb = ostg.tile([128, 512], F32, name="osb")
                nc.vector.tensor_tensor(osb, accs[ml_], x2[:, m, n * 512:(n + 1) * 512],
                                        op=OP.add)
                nc.vector.tensor_tensor(osb, osb, bfc2_sb[:, n * 512:(n + 1) * 512],
                                        op=OP.add)
                nc.sync.dma_start(out=out[m * 128:(m + 1) * 128, n * 512:(n + 1) * 512],
                                  in_=osb)


def make_masks(half):
    """[16, 128, 512] bf16 for qc = 2*half; slot1 uses the kt-4 shift identity."""
    m = np.zeros((16, 128, 512), np.float32)
    kk = np.arange(128)[:, None]
    qq = np.arange(512)[None, :]
    qc = 2 * half
    for kt in range(16):
        m[kt] = ((kt * 128 + kk) <= (qc * 512 + qq)).astype(np.float32)
    return m.astype(BF)


def make_in_maps(inputs):
    f32 = lambda a: np.asarray(a, dtype=np.float32)
    x = f32(inputs["x"])
    W_attn, b_attn = f32(inputs["W_attn"]), f32(inputs["b_attn"])
    W_proj, b_proj = f32(inputs["W_proj"]), f32(inputs["b_proj"])
    W_fc, b_fc = f32(inputs["W_fc"]), f32(inputs["b_fc"])
    W_fc2, b_fc2 = f32(inputs["W_fc2"]), f32(inputs["b_fc2"])
    g1, b1 = f32(inputs["ln1_g"]), f32(inputs["ln1_b"])
    g2, b2 = f32(inputs["ln2_g"]), f32(inputs["ln2_b"])

    bc = lambda v: np.ascontiguousarray(np.broadcast_to(v, (128, C)))
    shared = {
        "wqkv": W_attn.astype(BF), "b1col": b1[:, None].astype(BF),
        "wproj": W_proj.astype(BF), "wfc": W_fc.astype(BF),
        "wfc2": W_fc2.astype(BF),
        "bq_col": b_attn[:, None], "bq_row": b_attn[None, :],
        "g1col": g1[:, None],
        "bproj_bc": bc(b_proj), "g2bc": bc(g2), "b2bc": bc(b2),
        "bfc_col": b_fc[:, None], "bfc2_bc": bc(b_fc2),
    }
    masks = {h: make_masks(h) for h in range(2)}
    in_maps = []
    for c in range(8):
        b, half = c // 2, c % 2
        xb = x[b]
        in_maps.append(dict(
            shared,
            xT=np.ascontiguousarray(xb.T).astype(BF),
            qxT=np.ascontiguousarray(xb[half * TQ:(half + 1) * TQ].T).astype(BF),
            xn=xb,
            xr=np.ascontiguousarray(xb[half * TQ:(half + 1) * TQ]),
            masks=masks[half],
        ))
    return in_maps


def assemble_out(results):
    out = np.empty((B, T, C), np.float32)
    for c in range(8):
        b, half = c // 2, c % 2
        out[b, half * TQ:(half + 1) * TQ] = results[c]["out"]
    return out


_NC_CACHE = {}


def kernel(**inputs):
    if "nc" not in _NC_CACHE:
        _NC_CACHE["nc"] = build_nc()
    nc = _NC_CACHE["nc"]
    in_maps = make_in_maps(inputs)
    rr = run_bass_kernel_spmd(nc, in_maps, list(range(8)))
    return assemble_out(rr.results)



# revision 20
# speedup vs baseline: 1.3265x; 1.3265x over previous
"""Trainium2 Bass kernel for a dense transformer block (B=4, T=2048, C=1024, H=16).

Sharding: zero-collective. Each of the 8 cores owns (batch b, fold f):
  core c -> b = c//2, f = c%2.
Queries (1024 per core): fold0 owns token chunks {0, 3}, fold1 owns {1, 2}
(chunks of 512).  Keys are stored PERMUTED per core so the causal structure
looks identical on every core:
  fold0 key order: [c0 c1 c2 c3] (natural),  fold1: [c1 c0 c3 c2].
With q-slot0 = key-positions [0:512) and q-slot1 = positions [1536:2048),
both folds see: slot0 reads key tiles 0..7 (diagonal masks at kt 0..3),
slot1 reads tiles 0..15 (diagonal at kt 12..15).  Fully-invisible tiles
are zeroed via a per-core exp bias column (-1e9); the 4 distinct diagonal
masks are shared constants.  QK^T packs 2 heads per 512-cycle pass via
K=64 row-group concurrency (head A partitions 0:64, head B 64:128); exp
runs once over both heads [128, 2x512].  Softmax denominators ride a
ones-column in V; reciprocals are computed column-wise [128, 32] after a
DRAM-roundtrip transpose, then multiplied back into y row-broadcasts.

Host-side constant folding (weight-only transforms): ln1_g/ln2_g into
W_attn/W_fc, bias vectors b1@W'+b_attn / b2@W'+b_fc precomputed in numpy.
x is layer-normed once on-chip into bf16 xhat; qkv evictions are a single
cast+bias op.  b_proj is pre-added to the residual input; b_fc2 rides a
rank-1 matmul row.
"""
import sys
import math
import contextlib

for _p in ("/opt/trn_rl_repo", "/root/.axon_site/_ro/trn_rl_repo"):
    if _p not in sys.path:
        sys.path.append(_p)

import numpy as np
import ml_dtypes

import concourse.bass as bass
import concourse.bacc as bacc
import concourse.mybir as mybir
import concourse.tile as tile
from concourse.bass_utils import run_bass_kernel_spmd

F32 = mybir.dt.float32
BF16 = mybir.dt.bfloat16
AF = mybir.ActivationFunctionType
OP = mybir.AluOpType
BF = ml_dtypes.bfloat16

B, T, C, H = 4, 2048, 1024, 16
HD = C // H              # 64
EPS = 1e-5
TB = T                   # tokens per batch (2048)
TQ = T // 2              # query tokens per core (1024)
CK = C // 128            # 8 contraction k-tiles over C
NTT = TB // 128          # 16 token tiles per batch
NQT = TQ // 128          # 8 token tiles per core's queries
G = 4                    # head groups (4 heads each)
DG = 256                 # q/k/v cols per group
FC = 4 * C               # 4096
NGT = FC // 128          # 32 fc tiles
NEG = -1e9


def build_nc(dbg=False):
    nc = bacc.Bacc("TRN2", target_bir_lowering=False, debug=False, num_devices=8)

    dt_in = {
        "xT": ([C, TB], BF16), "xn": ([TB, C], BF16),
        "wqkv": ([C, 3 * C], BF16),
        "wproj": ([C, C], BF16), "wfc": ([C, FC], BF16),
        "wfc2": ([FC, C], BF16), "masks": ([4, 128, 2, 512], BF16),
        "xr": ([TQ, C], F32),
        "beta_col": ([3 * C, 1], F32),
        "betav_bc": ([128, C], F32),
        "ebias": ([128, 24], F32),
        "bfc_col": ([FC, 1], F32),
        "bfc2_row": ([1, C], BF16),
    }
    d = {k: nc.dram_tensor(k, sh, dt, kind="ExternalInput").ap()
         for k, (sh, dt) in dt_in.items()}
    out = nc.dram_tensor("out", [TQ, C], F32, kind="ExternalOutput").ap()
    if dbg:
        for k, sh, dt in [("dbg_xhat", [128, CK, TB], BF16),
                          ("dbg_q", [128, 2, TQ], BF16),
                          ("dbg_k", [128, 2, TB], BF16),
                          ("dbg_va", [128, NTT, 8, 65], BF16),
                          ("dbg_y", [128, 8, TQ], BF16),
                          ("dbg_x2", [128, NQT, C], F32)]:
            d[k] = nc.dram_tensor(k, sh, dt, kind="ExternalOutput").ap()

    with tile.TileContext(nc) as tc:
        with contextlib.ExitStack() as ctx:
            _build_body(nc, tc, ctx, d, out, dbg)
    nc.compile()
    return nc


def _build_body(nc, tc, ctx, d, out, dbg=False):
    pool = lambda name, bufs, **kw: ctx.enter_context(
        tc.tile_pool(name=name, bufs=bufs, **kw))

    cons = pool("cons", 1)
    stats = pool("stats", 3)
    ps = pool("ps", 2, space="PSUM")
    dram = pool("dram", 2, space="DRAM")
    yp = pool("yp", 1)

    eps_t = cons.tile([128, 1], F32)
    nc.vector.memset(eps_t, EPS)
    ident = cons.tile([128, 128], BF16)
    from concourse.masks import make_identity
    make_identity(nc, ident)

    beta_sb = cons.tile([128, 16], F32)
    nc.sync.dma_start(
        out=beta_sb,
        in_=d["beta_col"][0:2 * C, :].rearrange("(k p) o -> p (k o)", p=128))
    betav_sb = cons.tile([128, C], F32)
    nc.sync.dma_start(out=betav_sb, in_=d["betav_bc"])
    ebias_sb = cons.tile([128, 24], F32)
    nc.sync.dma_start(out=ebias_sb, in_=d["ebias"])
    masksb = cons.tile([128, 4, 2, 512], BF16)
    nc.sync.dma_start(out=masksb, in_=d["masks"].rearrange("m p h q -> p m h q"))

    # ---- phase 0: LN1 stats + xhat normalization (pipelined per 512-chunk) ----
    mid1 = contextlib.ExitStack()   # freed after attention (before proj)
    xh = mid1.enter_context(tc.tile_pool(name="xh", bufs=1))
    xhat = xh.tile([128, CK, TB], BF16)
    bc = mid1.enter_context(tc.tile_pool(name="bc", bufs=1))
    mu_bc = bc.tile([128, TB], BF16)
    r_bc = bc.tile([128, TB], BF16)
    mu_c = bc.tile([128, NTT], BF16)
    r_c = bc.tile([128, NTT], BF16)

    xTr = d["xT"].rearrange("(k p) t -> p k t", p=128)
    for ch in range(4):
        csl = slice(ch * 512, (ch + 1) * 512)
        nc.sync.dma_start(out=xhat[:, :, csl], in_=xTr[:, :, csl])
        for tl in range(4):
            tt = ch * 4 + tl
            xt_f = stats.tile([128, C], BF16, name="xt_f")
            nc.sync.dma_start(out=xt_f, in_=d["xn"][tt * 128:(tt + 1) * 128, :])
            st = stats.tile([128, 2, 6], F32, name="st")
            resh = xt_f.rearrange("p (n f) -> p n f", f=512)
            for i in range(2):
                nc.vector.bn_stats(out=st[:, i, :], in_=resh[:, i, :])
            mv = stats.tile([128, 2], F32, name="mv")
            nc.vector.bn_aggr(out=mv, in_=st)
            sd = stats.tile([128, 1], F32, name="sd")
            nc.scalar.activation(sd, mv[:, 1:2], AF.Sqrt, bias=eps_t)
            rf = stats.tile([128, 1], F32, name="rf")
            nc.vector.reciprocal(rf, sd)
            nc.vector.tensor_copy(r_c[:, tt:tt + 1], rf)
            nc.vector.tensor_copy(mu_c[:, tt:tt + 1], mv[:, 0:1])
        for col, dst in ((mu_c, mu_bc), (r_c, r_bc)):
            scr = dram.tile([4, 128], BF16, name="scr")
            nc.gpsimd.dma_start(out=scr.rearrange("t p -> p t"),
                                in_=col[:, ch * 4:(ch + 1) * 4])
            flat = scr.rearrange("t p -> (t p)").unsqueeze(0)
            nc.gpsimd.dma_start(out=dst[:, csl],
                                in_=flat.to_broadcast([128, 512]))
        for kt in range(CK):
            nc.vector.tensor_tensor(xhat[:, kt, csl], xhat[:, kt, csl],
                                    mu_bc[:, csl], op=OP.subtract)
            nc.vector.tensor_tensor(xhat[:, kt, csl], xhat[:, kt, csl],
                                    r_bc[:, csl], op=OP.mult)

    yT2 = yp.tile([128, 8, TQ], BF16)

    # ---- attention scope ----
    attn_ctx = contextlib.ExitStack()
    apool = lambda name, bufs, **kw: attn_ctx.enter_context(
        tc.tile_pool(name=name, bufs=bufs, **kw))
    wvp = apool("wvp", 1)
    wkq = apool("wkq", 2)
    vap = apool("vap", 2)
    qtp = apool("qtp", 2)
    ktp = apool("ktp", 2)
    pp = apool("pp", 6)
    dp = apool("dp", 2)
    rbp = apool("rbp", 4)
    otp = apool("otp", 2)
    ps2 = apool("ps2", 2, space="PSUM")
    psy = apool("psy", 1, space="PSUM")

    QSL = (slice(0, 512), slice(1536, 2048))   # query slots in position space

    def v_pass(slab):
        wv = wvp.tile([128, CK, 512], BF16, name="wv")
        for kt in range(CK):
            nc.sync.dma_start(
                out=wv[:, kt, :],
                in_=d["wqkv"][kt * 128:(kt + 1) * 128,
                              2 * C + slab * 512: 2 * C + (slab + 1) * 512])
        va = vap.tile([128, NTT, 8, 65], BF16, name="va")
        nc.vector.memset(va[:, :, :, 64:65], 1.0)
        bv = betav_sb[:, slab * 512:(slab + 1) * 512]
        for tt in range(NTT):
            psv = ps.tile([128, 512], F32, name="ps")
            for kt in range(CK):
                nc.tensor.matmul(psv, xhat[:, kt, tt * 128:(tt + 1) * 128],
                                 wv[:, kt, :], start=(kt == 0), stop=(kt == CK - 1))
            nc.vector.tensor_tensor(
                va[:, tt, :, 0:64],
                psv.rearrange("p (h f) -> p h f", f=64),
                bv.rearrange("p (h f) -> p h f", f=64), op=OP.add)
        return va

    va_slabs = {}
    for g in range(G):
        slab = g // 2
        if g % 2 == 0:
            va_slabs[slab] = v_pass(slab)
        va = va_slabs[slab]

        wt = wkq.tile([128, CK, 2, DG], BF16, name="wt")
        for kt in range(CK):
            for xi in range(2):                   # 0=q, 1=k
                nc.sync.dma_start(
                    out=wt[:, kt, xi, :],
                    in_=d["wqkv"][kt * 128:(kt + 1) * 128,
                                  xi * C + g * DG: xi * C + (g + 1) * DG])

        qT = qtp.tile([128, 2, TQ], BF16, name="qT")
        kT = ktp.tile([128, 2, TB], BF16, name="kT")

        for j in range(2):
            for slot in range(2):
                psq = ps.tile([128, 512], F32, name="ps")
                for kt in range(CK):
                    nc.tensor.matmul(psq, wt[:, kt, 0, j * 128:(j + 1) * 128],
                                     xhat[:, kt, QSL[slot]],
                                     start=(kt == 0), stop=(kt == CK - 1))
                nc.vector.tensor_scalar_add(
                    qT[:, j, slot * 512:(slot + 1) * 512], in0=psq,
                    scalar1=beta_sb[:, 2 * g + j: 2 * g + j + 1])
        for j in range(2):
            for ch in range(4):
                psk = ps.tile([128, 512], F32, name="ps")
                for kt in range(CK):
                    nc.tensor.matmul(psk, wt[:, kt, 1, j * 128:(j + 1) * 128],
                                     xhat[:, kt, ch * 512:(ch + 1) * 512],
                                     start=(kt == 0), stop=(kt == CK - 1))
                nc.vector.tensor_scalar_add(
                    kT[:, j, ch * 512:(ch + 1) * 512], in0=psk,
                    scalar1=beta_sb[:, 8 + 2 * g + j: 8 + 2 * g + j + 1])

        if dbg and g == 0:
            nc.sync.dma_start(out=d["dbg_xhat"], in_=xhat)
            nc.sync.dma_start(out=d["dbg_q"], in_=qT)
            nc.sync.dma_start(out=d["dbg_k"], in_=kT)
            nc.sync.dma_start(out=d["dbg_va"], in_=va)

        dscr = dram.tile([4, 2, 512], F32, name="dscr")
        for j in range(2):
            hA = (g % 2) * 4 + 2 * j   # slab-relative head index (even)
            for slot in range(2):
                nkt = 8 if slot == 0 else 16
                pya = psy.tile([65, 512], F32, name="pya")
                pyb = psy.tile([65, 512], F32, name="pyb")
                for kt in range(nkt):
                    p2 = ps2.tile([128, 2, 512], F32, name="p2")
                    nc.tensor.matmul(p2[:, 0, :],
                                     kT[0:64, j, kt * 128:(kt + 1) * 128],
                                     qT[0:64, j, slot * 512:(slot + 1) * 512],
                                     start=True, stop=True)
                    nc.tensor.matmul(p2[:, 1, :],
                                     kT[64:128, j, kt * 128:(kt + 1) * 128],
                                     qT[64:128, j, slot * 512:(slot + 1) * 512],
                                     start=True, stop=True)
                    P2 = pp.tile([128, 2, 512], BF16, name="P2")
                    epos = kt if slot == 0 else 8 + kt
                    nc.scalar.activation(
                        P2.rearrange("p h q -> p (h q)"),
                        p2.rearrange("p h q -> p (h q)"),
                        AF.Exp, scale=1.0 / math.sqrt(HD),
                        bias=ebias_sb[:, epos:epos + 1])
                    mpos = kt if slot == 0 else kt - 12
                    if 0 <= mpos < 4:
                        nc.vector.tensor_mul(P2, P2, masksb[:, mpos, :, :])
                    nc.tensor.matmul(pya, va[:, kt, hA, :], P2[:, 0, :],
                                     start=(kt == 0), stop=(kt == nkt - 1))
                    nc.tensor.matmul(pyb, va[:, kt, hA + 1, :], P2[:, 1, :],
                                     start=(kt == 0), stop=(kt == nkt - 1))
                # evict unnormalized y (bf16) + denominator rows
                pair = g * 2 + j
                qs = slice(slot * 512, (slot + 1) * 512)
                nc.vector.tensor_copy(yT2[0:64, pair, qs], pya[0:64, :])
                otmp = otp.tile([64, 512], BF16, name="otmp")
                nc.vector.tensor_copy(otmp, pyb[0:64, :])
                nc.sync.dma_start(out=yT2[64:128, pair, qs], in_=otmp)
                dsb = dp.tile([128, 2, 512], F32, name="dsb")
                nc.vector.tensor_copy(dsb[64:65, 0, :], pya[64:65, :])
                nc.vector.tensor_copy(dsb[64:65, 1, :], pyb[64:65, :])
                rp = j * 2 + slot
                nc.gpsimd.dma_start(out=dscr[rp, :, :].unsqueeze(0),
                                    in_=dsb[64:65, :, :])

        # group-end: columnwise reciprocal of the 8 denominator rows
        dcol = dp.tile([128, 32], F32, name="dcol")
        nc.gpsimd.dma_start(out=dcol, in_=dscr.rearrange("r h q -> (r h q)")
                            .rearrange("(i p) -> p i", p=128))
        rcol = dp.tile([128, 32], BF16, name="rcol")
        with nc.allow_low_precision(reason="softmax denom reciprocal in bf16"):
            nc.vector.reciprocal(rcol, dcol)
        rscr = dram.tile([8, 512], BF16, name="rscr")
        nc.gpsimd.dma_start(out=rscr.rearrange("r q -> (r q)")
                            .rearrange("(i p) -> p i", p=128), in_=rcol)
        for j in range(2):
            for slot in range(2):
                pair = g * 2 + j
                qs = slice(slot * 512, (slot + 1) * 512)
                for h in range(2):
                    ridx = (j * 2 + slot) * 2 + h
                    rb_t = rbp.tile([128, 512], BF16, name="rb_t")
                    psl = slice(h * 64, h * 64 + 64)
                    nc.gpsimd.dma_start(
                        out=rb_t[psl, :],
                        in_=rscr[ridx:ridx + 1, :].to_broadcast([64, 512]))
                    nc.vector.tensor_mul(yT2[psl, pair, qs],
                                         yT2[psl, pair, qs], rb_t[psl, :])

    if dbg:
        nc.sync.dma_start(out=d["dbg_y"], in_=yT2)
    attn_ctx.close()
    mid1.close()

    # ---- proj + residual ----
    mlp = pool("mlp", 1)
    mstr = pool("mstr", 2)
    wstream = pool("wstream", 2)
    c2 = pool("c2", 1)
    x2 = mlp.tile([128, NQT, C], F32)

    bfc_sb = c2.tile([128, NGT], F32)
    nc.sync.dma_start(out=bfc_sb,
                      in_=d["bfc_col"].rearrange("(k p) o -> p (k o)", p=128))
    ones_row = c2.tile([1, 128], BF16)
    nc.vector.memset(ones_row, 1.0)
    bfc2_sb = c2.tile([1, C], BF16)
    nc.sync.dma_start(out=bfc2_sb, in_=d["bfc2_row"])

    wpjp = pool("wpjp", 1)
    wpj = []
    for kt in range(CK):
        w = wpjp.tile([128, C], BF16, name=f"wpj{kt}")
        nc.sync.dma_start(out=w, in_=d["wproj"][kt * 128:(kt + 1) * 128, :])
        wpj.append(w)

    for m in range(NQT):
        xr_t = mstr.tile([128, C], F32, name="xr_t")
        nc.sync.dma_start(out=xr_t, in_=d["xr"][m * 128:(m + 1) * 128, :])
        for n in range(2):
            psp = ps.tile([128, 512], F32, name="ps")
            for kt in range(CK):
                nc.tensor.matmul(psp, yT2[:, kt, m * 128:(m + 1) * 128],
                                 wpj[kt][:, n * 512:(n + 1) * 512],
                                 start=(kt == 0), stop=(kt == CK - 1))
            sl = slice(n * 512, (n + 1) * 512)
            nc.vector.tensor_tensor(x2[:, m, sl], psp, xr_t[:, sl], op=OP.add)

    if dbg:
        nc.sync.dma_start(out=d["dbg_x2"], in_=x2)

    # ---- LN2 + transpose (g2/b2 folded into wfc/beta2 host-side) ----
    hT = mlp.tile([128, CK, TQ], BF16)
    for m in range(NQT):
        st = stats.tile([128, 2, 6], F32, name="st")
        resh = x2[:, m, :].rearrange("p (n f) -> p n f", f=512)
        for i in range(2):
            nc.vector.bn_stats(out=st[:, i, :], in_=resh[:, i, :])
        mv = stats.tile([128, 2], F32, name="mv")
        nc.vector.bn_aggr(out=mv, in_=st)
        sd = stats.tile([128, 1], F32, name="sd")
        nc.scalar.activation(sd, mv[:, 1:2], AF.Sqrt, bias=eps_t)
        r2 = stats.tile([128, 1], F32, name="r2")
        nc.vector.reciprocal(r2, sd)
        hmb = mstr.tile([128, C], BF16, name="hmb")
        nc.vector.tensor_scalar(hmb, in0=x2[:, m, :], scalar1=mv[:, 0:1],
                                scalar2=r2, op0=OP.subtract, op1=OP.mult)
        for ck in range(CK):
            pst = ps.tile([128, 512], F32, name="ps")
            pstv = pst.bitcast(BF16)[:, 0:128]
            nc.tensor.transpose(pstv, hmb[:, ck * 128:(ck + 1) * 128], ident)
            nc.scalar.copy(hT[:, ck, m * 128:(m + 1) * 128], pstv)

    # ---- MLP ----
    hid = mlp.tile([128, NGT, 512], BF16, name="hid")
    psacc = pool("psacc", 4, space="PSUM")
    ostg = pool("ostg", 3)
    for th in range(2):
        for gtg in range(NGT // 4):
            wfcg = wstream.tile([128, CK, 512], BF16, name="wfcg")
            for kt in range(CK):
                nc.sync.dma_start(out=wfcg[:, kt, :],
                                  in_=d["wfc"][kt * 128:(kt + 1) * 128,
                                               gtg * 512:(gtg + 1) * 512])
            for gi in range(4):
                gt = gtg * 4 + gi
                psf = ps.tile([128, 512], F32, name="ps")
                for kt in range(CK):
                    nc.tensor.matmul(psf, wfcg[:, kt, gi * 128:(gi + 1) * 128],
                                     hT[:, kt, th * 512:(th + 1) * 512],
                                     start=(kt == 0), stop=(kt == CK - 1))
                nc.scalar.activation(hid[:, gt, :], psf, AF.Gelu,
                                     bias=bfc_sb[:, gt:gt + 1])
        for n in range(2):
            accs = [psacc.tile([128, 512], F32, name="acc") for _ in range(4)]
            for gkt in range(NGT):
                wf2 = wstream.tile([128, 512], BF16, name="wf2")
                nc.sync.dma_start(out=wf2,
                                  in_=d["wfc2"][gkt * 128:(gkt + 1) * 128,
                                                n * 512:(n + 1) * 512])
                for ml_ in range(4):
                    nc.tensor.matmul(accs[ml_],
                                     hid[:, gkt, ml_ * 128:(ml_ + 1) * 128],
                                     wf2, start=(gkt == 0), stop=False)
            for ml_ in range(4):
                m = th * 4 + ml_
                nc.tensor.matmul(accs[ml_], ones_row,
                                 bfc2_sb[:, n * 512:(n + 1) * 512],
                                 start=False, stop=True)
                osb = ostg.tile([128, 512], F32, name="osb")
                nc.vector.tensor_tensor(osb, accs[ml_],
                                        x2[:, m, n * 512:(n + 1) * 512], op=OP.add)
                nc.sync.dma_start(out=out[m * 128:(m + 1) * 128,
                                          n * 512:(n + 1) * 512],
                                  in_=osb)


def make_masks():
    """[4, 128, 2, 512] bf16 diagonal masks, duplicated across the head dim."""
    m = np.zeros((4, 128, 512), np.float32)
    kk = np.arange(128)[:, None]
    qq = np.arange(512)[None, :]
    for j in range(4):
        m[j] = ((j * 128 + kk) <= qq).astype(np.float32)
    m2 = np.repeat(m[:, :, None, :], 2, axis=2)
    return np.ascontiguousarray(m2).astype(BF)


def make_in_maps(inputs):
    f32 = lambda a: np.asarray(a, dtype=np.float32)
    x = f32(inputs["x"])
    W_attn, b_attn = f32(inputs["W_attn"]), f32(inputs["b_attn"])
    W_proj, b_proj = f32(inputs["W_proj"]), f32(inputs["b_proj"])
    W_fc, b_fc = f32(inputs["W_fc"]), f32(inputs["b_fc"])
    W_fc2, b_fc2 = f32(inputs["W_fc2"]), f32(inputs["b_fc2"])
    g1, b1 = f32(inputs["ln1_g"]), f32(inputs["ln1_b"])
    g2, b2 = f32(inputs["ln2_g"]), f32(inputs["ln2_b"])

    # host-side constant folding (weight-only transforms)
    Wq = W_attn * g1[:, None]                        # [C, 3C]
    beta = b1 @ Wq + b_attn                          # [3C]
    Wf = W_fc * g2[:, None]                          # [C, 4C]
    beta2 = b2 @ Wf + b_fc                           # [4C]

    masks = make_masks()
    ebias = np.zeros((2, 128, 24), np.float32)
    ebias[0, :, 4:8] = NEG          # fold0: slot0 kt4..7 invisible
    ebias[1, :, 16:20] = NEG        # fold1: slot1 kt8..11 invisible

    shared = {
        "wqkv": Wq.astype(BF),
        "wproj": W_proj.astype(BF), "wfc": Wf.astype(BF),
        "wfc2": W_fc2.astype(BF), "masks": masks,
        "beta_col": beta[:, None],
        "betav_bc": np.ascontiguousarray(
            np.broadcast_to(beta[2 * C:], (128, C))),
        "bfc_col": beta2[:, None],
        "bfc2_row": b_fc2[None, :].astype(BF),
    }
    in_maps = []
    chunk_orders = ([0, 1, 2, 3], [1, 0, 3, 2])
    for c in range(8):
        b, f = c // 2, c % 2
        order = chunk_orders[f]
        xp = np.concatenate([x[b, 512 * ci:512 * (ci + 1)] for ci in order])
        xr = np.concatenate([xp[0:512], xp[1536:2048]]) + b_proj[None, :]
        in_maps.append(dict(
            shared,
            xT=np.ascontiguousarray(xp.T).astype(BF),
            xn=xp.astype(BF),
            xr=np.ascontiguousarray(xr),
            ebias=ebias[f],
        ))
    return in_maps


def assemble_out(results):
    out = np.empty((B, T, C), np.float32)
    for c in range(8):
        b, f = c // 2, c % 2
        r = results[c]["out"]
        if f == 0:
            out[b, 0:512] = r[0:512]
            out[b, 1536:2048] = r[512:1024]
        else:
            out[b, 512:1024] = r[0:512]
            out[b, 1024:1536] = r[512:1024]
    return out


_NC_CACHE = {}


def kernel(**inputs):
    if "nc" not in _NC_CACHE:
        _NC_CACHE["nc"] = build_nc()
    nc = _NC_CACHE["nc"]
    in_maps = make_in_maps(inputs)
    rr = run_bass_kernel_spmd(nc, in_maps, list(range(8)))
    return assemble_out(rr.results)


# revision 56
# speedup vs baseline: 1.7668x; 1.3320x over previous
"""Trainium2 Bass kernel for a dense transformer block (B=4, T=2048, C=1024, H=16).

Sharding: zero-collective. Each of the 8 cores owns (batch b, fold f):
  core c -> b = c//2, f = c%2.
Queries (1024 per core): fold0 owns token chunks {0, 3}, fold1 owns {1, 2}
(chunks of 512).  Keys are stored PERMUTED per core so the causal structure
looks identical on every core:
  fold0 key order: [c0 c1 c2 c3] (natural),  fold1: [c1 c0 c3 c2].
With q-slot0 = key-positions [0:512) and q-slot1 = positions [1536:2048),
both folds see: slot0 reads key tiles 0..7 (diagonal masks at kt 0..3),
slot1 reads tiles 0..15 (diagonal at kt 12..15).  Fully-invisible tiles
are zeroed via a per-core exp bias column (-1e9); the 4 distinct diagonal
masks are shared constants.  QK^T packs 2 heads per 512-cycle pass via
K=64 row-group concurrency (head A partitions 0:64, head B 64:128); exp
runs once over both heads [128, 2x512].  Softmax denominators ride a
ones-column in V; reciprocals are computed column-wise [128, 32] after a
DRAM-roundtrip transpose, then multiplied back into y row-broadcasts.

Host-side constant folding (weight-only transforms): ln1_g/ln2_g into
W_attn/W_fc, bias vectors b1@W'+b_attn / b2@W'+b_fc precomputed in numpy.
x is layer-normed once on-chip into bf16 xhat; qkv evictions are a single
cast+bias op.  b_proj is pre-added to the residual input; b_fc2 rides a
rank-1 matmul row.
"""
import sys
import math
import contextlib

for _p in ("/opt/trn_rl_repo", "/root/.axon_site/_ro/trn_rl_repo"):
    if _p not in sys.path:
        sys.path.append(_p)

import numpy as np
import ml_dtypes

import concourse.bass as bass
import concourse.bacc as bacc
import concourse.mybir as mybir
import concourse.tile as tile
from concourse.bass_utils import run_bass_kernel_spmd

F32 = mybir.dt.float32
BF16 = mybir.dt.bfloat16
AF = mybir.ActivationFunctionType
OP = mybir.AluOpType
BF = ml_dtypes.bfloat16

B, T, C, H = 4, 2048, 1024, 16
HD = C // H              # 64
EPS = 1e-5
TB = T                   # tokens per batch (2048)
TQ = T // 2              # query tokens per core (1024)
CK = C // 128            # 8 contraction k-tiles over C
NTT = TB // 128          # 16 token tiles per batch
NQT = TQ // 128          # 8 token tiles per core's queries
G = 4                    # head groups (4 heads each)
DG = 256                 # q/k/v cols per group
FC = 4 * C               # 4096
NGT = FC // 128          # 32 fc tiles
NEG = -1e9


def build_nc(dbg=False):
    nc = bacc.Bacc("TRN2", target_bir_lowering=False, debug=False, num_devices=8)

    # all tensors pre-arranged host-side to be contiguous per partition
    dt_in = {
        "xT": ([128, 4, CK, 512], BF16),       # [p, ch, k, t]
        "xn": ([128, 4, 4, C], BF16),          # [p, ch, tl, c]
        "wvw": ([128, 2, CK, 512], BF16),      # [p, slab, k, c]
        "wkqw": ([128, G, CK, 2, DG], BF16),   # [p, g, k, xi, c]
        "wproj": ([128, CK, C], BF16),
        "wfc": ([128, 8, CK, 512], BF16),      # [p, gtg, k, c]
        "wfc2": ([128, 2, 8, 4, 512], BF16),   # [p, n, gkb, gi, c]
        "masks": ([128, 4, 2, 512], BF16),
        "xr": ([128, NQT, C], F32),            # [p, m, c]
        "beta_col": ([128, 16], F32),
        "betav_bc": ([128, C], F32),
        "ebias": ([128, 24], F32),
        "bfc_col": ([128, NGT], F32),
        "bfc2_row": ([1, C], BF16),
    }
    d = {k: nc.dram_tensor(k, sh, dt, kind="ExternalInput").ap()
         for k, (sh, dt) in dt_in.items()}
    out = nc.dram_tensor("out", [TQ, C], F32, kind="ExternalOutput").ap()
    if dbg:
        for k, sh, dt in [("dbg_xhat", [128, CK, TB], BF16),
                          ("dbg_q", [128, 2, TQ], BF16),
                          ("dbg_k", [128, 2, TB], BF16),
                          ("dbg_va", [128, NTT, 8, 65], BF16),
                          ("dbg_y", [128, 8, TQ], BF16),
                          ("dbg_x2", [128, NQT, C], F32)]:
            d[k] = nc.dram_tensor(k, sh, dt, kind="ExternalOutput").ap()

    with tile.TileContext(nc) as tc:
        with contextlib.ExitStack() as ctx:
            _build_body(nc, tc, ctx, d, out, dbg)
    nc.compile()
    return nc


def _build_body(nc, tc, ctx, d, out, dbg=False):
    pool = lambda name, bufs, **kw: ctx.enter_context(
        tc.tile_pool(name=name, bufs=bufs, **kw))

    cons = pool("cons", 1)
    stats = pool("stats", 3)
    ps = pool("ps", 2, space="PSUM")
    dram = pool("dram", 2, space="DRAM")
    yp = pool("yp", 1)
    wpjp = pool("wpjp", 1)

    eps_t = cons.tile([128, 1], F32)
    nc.vector.memset(eps_t, EPS)
    ident = cons.tile([128, 128], BF16)
    from concourse.masks import make_identity
    make_identity(nc, ident)

    beta_sb = cons.tile([128, 16], F32)
    nc.sync.dma_start(out=beta_sb, in_=d["beta_col"])
    betav_sb = cons.tile([128, C], F32)
    nc.sync.dma_start(out=betav_sb, in_=d["betav_bc"])
    ebias_sb = cons.tile([128, 24], F32)
    nc.sync.dma_start(out=ebias_sb, in_=d["ebias"])
    masksb = cons.tile([128, 4, 2, 512], BF16)
    nc.sync.dma_start(out=masksb, in_=d["masks"])

    # ---- phase 0: LN1 stats + xhat normalization (pipelined per 512-chunk) ----
    wpj_t = wpjp.tile([128, CK, C], BF16)

    mid1 = contextlib.ExitStack()   # freed after attention (before proj)
    xh = mid1.enter_context(tc.tile_pool(name="xh", bufs=1))
    # per-chunk tiles so consumers only depend on their own chunk's normalize
    xhat_c = [xh.tile([128, CK, 512], BF16, name=f"xhat{ch}") for ch in range(4)]
    bc = mid1.enter_context(tc.tile_pool(name="bc", bufs=2))

    # ---- attention-scope pools (created early so weight prefetch can start) ----
    attn_ctx = contextlib.ExitStack()
    apool = lambda name, bufs, **kw: attn_ctx.enter_context(
        tc.tile_pool(name=name, bufs=bufs, **kw))
    wvp = apool("wvp", 1)
    wkq = apool("wkq", 2)
    vap = apool("vap", 2)
    qtp = apool("qtp", 2)
    ktp = apool("ktp", 2)
    pp = apool("pp", 6)
    dp = apool("dp", 2)
    rbp = apool("rbp", 4)
    otp = apool("otp", 2)
    ps2 = apool("ps2", 2, space="PSUM")
    psy = apool("psy", 1, space="PSUM")

    def load_wv(slab):
        wv = wvp.tile([128, CK, 512], BF16, name="wv")
        nc.sync.dma_start(out=wv, in_=d["wvw"][:, slab])
        return wv

    def load_wt(g):
        wt = wkq.tile([128, CK, 2, DG], BF16, name="wt")
        nc.sync.dma_start(out=wt, in_=d["wkqw"][:, g])
        return wt

    pre_wv = load_wv(0)
    pre_wt = {0: load_wt(0), 1: load_wt(1)}

    for ch in range(4):
        xhat = xhat_c[ch]
        nc.scalar.dma_start(out=xhat, in_=d["xT"][:, ch])
        xt_f = stats.tile([128, 4, C], BF16, name="xt_f")
        nc.scalar.dma_start(out=xt_f, in_=d["xn"][:, ch])
        mr_cc = stats.tile([128, 8], BF16, name="mr_cc")
        for tl in range(4):
            st = stats.tile([128, 2, 6], F32, name="st")
            resh = xt_f[:, tl, :].rearrange("p (n f) -> p n f", f=512)
            for i in range(2):
                nc.vector.bn_stats(out=st[:, i, :], in_=resh[:, i, :])
            mv = stats.tile([128, 2], F32, name="mv")
            nc.vector.bn_aggr(out=mv, in_=st)
            sd = stats.tile([128, 1], F32, name="sd")
            nc.scalar.activation(sd, mv[:, 1:2], AF.Sqrt, bias=eps_t)
            rf = stats.tile([128, 1], F32, name="rf")
            nc.vector.reciprocal(rf, sd)
            nc.vector.tensor_copy(mr_cc[:, tl:tl + 1], mv[:, 0:1])
            nc.vector.tensor_copy(mr_cc[:, 4 + tl:5 + tl], rf)
        # transpose [128, 8] -> [8, 128] rows, roundtrip to DRAM, bcast-read
        pst = ps.tile([128, 512], F32, name="ps")
        pstv = pst.bitcast(BF16)[0:8, 0:128]
        nc.tensor.transpose(pstv, mr_cc, ident)
        srow = stats.tile([8, 128], BF16, name="srow")
        nc.vector.tensor_copy(srow, pstv)
        mscr = dram.tile([8, 128], BF16, name="mscr")
        nc.sync.dma_start(out=mscr, in_=srow)
        mflat = mscr.rearrange("r q -> (r q)").unsqueeze(0)
        mu_bc = bc.tile([128, 512], BF16, name="mu_bc")
        r_bc = bc.tile([128, 512], BF16, name="r_bc")
        nc.sync.dma_start(out=mu_bc,
                          in_=mflat[:, 0:512].to_broadcast([128, 512]))
        nc.sync.dma_start(out=r_bc,
                          in_=mflat[:, 512:1024].to_broadcast([128, 512]))
        for kt in range(CK):
            nc.vector.tensor_tensor(xhat[:, kt, :], xhat[:, kt, :],
                                    mu_bc, op=OP.subtract)
            nc.vector.tensor_tensor(xhat[:, kt, :], xhat[:, kt, :],
                                    r_bc, op=OP.mult)

    yT2_p = [yp.tile([128, TQ], BF16, name=f"yT2_{p}") for p in range(8)]

    # ---- attention ----

    def v_pass(slab):
        wv = pre_wv if slab == 0 else load_wv(slab)
        va_c = [vap.tile([128, 4, 8, 65], BF16, name=f"va{ch}")
                for ch in range(4)]
        bv = betav_sb[:, slab * 512:(slab + 1) * 512]
        for ch in range(4):
            nc.vector.memset(va_c[ch][:, :, :, 64:65], 1.0)
            for tl in range(4):
                psv = ps.tile([128, 512], F32, name="ps")
                xs = xhat_c[ch]
                tsl = slice(tl * 128, (tl + 1) * 128)
                for kt in range(CK):
                    nc.tensor.matmul(psv, xs[:, kt, tsl], wv[:, kt, :],
                                     start=(kt == 0), stop=(kt == CK - 1))
                nc.vector.tensor_tensor(
                    va_c[ch][:, tl, :, 0:64],
                    psv.rearrange("p (h f) -> p h f", f=64),
                    bv.rearrange("p (h f) -> p h f", f=64), op=OP.add)
        return va_c

    va_slabs = {}
    pending_norm = None
    for g in range(G):
        slab = g // 2
        if g % 2 == 0:
            va_slabs[slab] = v_pass(slab)
        va = va_slabs[slab]

        wt = pre_wt.pop(g) if g in pre_wt else load_wt(g)
        if g == 3:
            nc.sync.dma_start(out=wpj_t, in_=d["wproj"])

        qT_s = [qtp.tile([128, 2, 512], BF16, name=f"qT{slot}")
                for slot in range(2)]
        kT_c = [ktp.tile([128, 2, 512], BF16, name=f"kT{ch}")
                for ch in range(4)]

        def k_ev(j, ch):
            psk = ps.tile([128, 512], F32, name="ps")
            for kt in range(CK):
                nc.tensor.matmul(psk, wt[:, kt, 1, j * 128:(j + 1) * 128],
                                 xhat_c[ch][:, kt, :],
                                 start=(kt == 0), stop=(kt == CK - 1))
            nc.vector.tensor_scalar_add(
                kT_c[ch][:, j, :], in0=psk,
                scalar1=beta_sb[:, 8 + 2 * g + j: 8 + 2 * g + j + 1])

        def q_ev(j, slot):
            psq = ps.tile([128, 512], F32, name="ps")
            xs = xhat_c[0 if slot == 0 else 3]
            for kt in range(CK):
                nc.tensor.matmul(psq, wt[:, kt, 0, j * 128:(j + 1) * 128],
                                 xs[:, kt, :],
                                 start=(kt == 0), stop=(kt == CK - 1))
            nc.vector.tensor_scalar_add(
                qT_s[slot][:, j, :], in0=psq,
                scalar1=beta_sb[:, 2 * g + j: 2 * g + j + 1])

        for ch in (0, 1):
            k_ev(0, ch); k_ev(1, ch)
        q_ev(0, 0); q_ev(1, 0)
        for ch in (2, 3):
            k_ev(0, ch); k_ev(1, ch)
        q_ev(0, 1); q_ev(1, 1)

        if dbg and g == 0:
            for _ch in range(4):
                nc.sync.dma_start(
                    out=d["dbg_xhat"][:, :, _ch * 512:(_ch + 1) * 512],
                    in_=xhat_c[_ch])


        for j in range(2):
            hA = (g % 2) * 4 + 2 * j   # slab-relative head index (even)
            for slot in range(2):
                dscr = dram.tile([2, 512], BF16, name="dscr")
                nkt = 8 if slot == 0 else 16
                pya = psy.tile([65, 512], F32, name="pya")
                pyb = psy.tile([65, 512], F32, name="pyb")
                for kt in range(nkt):
                    p2 = ps2.tile([128, 2, 512], F32, name="p2")
                    kts = kT_c[kt // 4]
                    ksl = slice((kt % 4) * 128, (kt % 4 + 1) * 128)
                    nc.tensor.matmul(p2[:, 0, :],
                                     kts[0:64, j, ksl],
                                     qT_s[slot][0:64, j, :],
                                     start=True, stop=True)
                    nc.tensor.matmul(p2[:, 1, :],
                                     kts[64:128, j, ksl],
                                     qT_s[slot][64:128, j, :],
                                     start=True, stop=True)
                    P2 = pp.tile([128, 2, 512], BF16, name="P2")
                    epos = kt if slot == 0 else 8 + kt
                    nc.scalar.activation(
                        P2.rearrange("p h q -> p (h q)"),
                        p2.rearrange("p h q -> p (h q)"),
                        AF.Exp, scale=1.0 / math.sqrt(HD),
                        bias=ebias_sb[:, epos:epos + 1])
                    mpos = kt if slot == 0 else kt - 12
                    if 0 <= mpos < 4:
                        nc.vector.tensor_mul(P2, P2, masksb[:, mpos, :, :])
                    vas = va[kt // 4]
                    nc.tensor.matmul(pya, vas[:, kt % 4, hA, :], P2[:, 0, :],
                                     start=(kt == 0), stop=(kt == nkt - 1))
                    nc.tensor.matmul(pyb, vas[:, kt % 4, hA + 1, :], P2[:, 1, :],
                                     start=(kt == 0), stop=(kt == nkt - 1))
                # evict unnormalized y (bf16) + denominator rows
                pair = g * 2 + j
                qs = slice(slot * 512, (slot + 1) * 512)
                nc.vector.tensor_copy(yT2_p[pair][0:64, qs], pya[0:64, :])
                otmp = otp.tile([64, 512], BF16, name="otmp")
                nc.vector.tensor_copy(otmp, pyb[0:64, :])
                nc.gpsimd.dma_start(out=yT2_p[pair][64:128, qs], in_=otmp)
                dsb = dp.tile([128, 2, 512], BF16, name="dsb")
                nc.vector.tensor_copy(dsb[64:65, 0, :], pya[64:65, :])
                nc.vector.tensor_copy(dsb[64:65, 1, :], pyb[64:65, :])
                nc.gpsimd.dma_start(out=dscr.rearrange("r q -> (r q)").unsqueeze(0),
                                    in_=dsb[64:65, :, :])

                # columnwise reciprocal of this slot's 2 denominator rows
                # (flat[p*8+i] layout keeps every DMA contiguous per partition)
                dcol = dp.tile([128, 8], BF16, name="dcol")
                nc.gpsimd.dma_start(out=dcol, in_=dscr.rearrange("r q -> (r q)")
                                    .rearrange("(p i) -> p i", p=128))
                rcol = dp.tile([128, 8], BF16, name="rcol")
                with nc.allow_low_precision(reason="softmax denom recip bf16"):
                    nc.vector.reciprocal(rcol, dcol)
                rscr = dram.tile([2, 512], BF16, name="rscr")
                nc.gpsimd.dma_start(out=rscr.rearrange("r q -> (r q)")
                                    .rearrange("(p i) -> p i", p=128), in_=rcol)

                def norm_muls(pair_, slot_, rscr_):
                    qs = slice(slot_ * 512, (slot_ + 1) * 512)
                    for h in range(2):
                        rb_t = rbp.tile([128, 512], BF16, name="rb_t")
                        psl = slice(h * 64, h * 64 + 64)
                        nc.sync.dma_start(
                            out=rb_t[psl, :],
                            in_=rscr_[h:h + 1, :].to_broadcast([64, 512]))
                        nc.vector.tensor_mul(yT2_p[pair_][psl, qs],
                                             yT2_p[pair_][psl, qs], rb_t[psl, :])

                # defer the previous slot's yT2 normalization so the in-order
                # vector queue never stalls on the denominator DMA roundtrip
                if pending_norm is not None:
                    pending_norm()
                pending_norm = (lambda p_=g * 2 + j, s_=slot, r_=rscr:
                                norm_muls(p_, s_, r_))

    pending_norm()
    if dbg:
        for _p in range(8):
            nc.sync.dma_start(out=d["dbg_y"][:, _p, :], in_=yT2_p[_p])
    attn_ctx.close()
    mid1.close()

    # ---- proj + residual ----
    mlp = pool("mlp", 1)
    mstr = pool("mstr", 2)
    wstream = pool("wstream", 3)
    c2 = pool("c2", 1)
    x2 = mlp.tile([128, NQT, C], F32)

    bfc_sb = c2.tile([128, NGT], F32)
    nc.sync.dma_start(out=bfc_sb, in_=d["bfc_col"])
    ones_row = c2.tile([1, 128], BF16)
    nc.vector.memset(ones_row, 1.0)
    bfc2_sb = c2.tile([1, C], BF16)
    nc.sync.dma_start(out=bfc2_sb, in_=d["bfc2_row"])

    for m in range(NQT):
        xr_t = mstr.tile([128, C], F32, name="xr_t")
        nc.sync.dma_start(out=xr_t, in_=d["xr"][:, m])
        for n in range(2):
            psp = ps.tile([128, 512], F32, name="ps")
            for kt in range(CK):
                nc.tensor.matmul(psp, yT2_p[kt][:, m * 128:(m + 1) * 128],
                                 wpj_t[:, kt, n * 512:(n + 1) * 512],
                                 start=(kt == 0), stop=(kt == CK - 1))
            sl = slice(n * 512, (n + 1) * 512)
            nc.vector.tensor_tensor(x2[:, m, sl], psp, xr_t[:, sl], op=OP.add)

    if dbg:
        nc.sync.dma_start(out=d["dbg_x2"], in_=x2)

    # ---- LN2 + transpose (g2/b2 folded into wfc/beta2 host-side) ----
    hT = mlp.tile([128, CK, TQ], BF16)
    for m in range(NQT):
        st = stats.tile([128, 2, 6], F32, name="st")
        resh = x2[:, m, :].rearrange("p (n f) -> p n f", f=512)
        for i in range(2):
            nc.vector.bn_stats(out=st[:, i, :], in_=resh[:, i, :])
        mv = stats.tile([128, 2], F32, name="mv")
        nc.vector.bn_aggr(out=mv, in_=st)
        sd = stats.tile([128, 1], F32, name="sd")
        nc.scalar.activation(sd, mv[:, 1:2], AF.Sqrt, bias=eps_t)
        r2 = stats.tile([128, 1], F32, name="r2")
        nc.vector.reciprocal(r2, sd)
        hmb = mstr.tile([128, C], BF16, name="hmb")
        nc.vector.tensor_scalar(hmb, in0=x2[:, m, :], scalar1=mv[:, 0:1],
                                scalar2=r2, op0=OP.subtract, op1=OP.mult)
        for ck in range(CK):
            pst = ps.tile([128, 512], F32, name="ps")
            pstv = pst.bitcast(BF16)[:, 0:128]
            nc.tensor.transpose(pstv, hmb[:, ck * 128:(ck + 1) * 128], ident)
            nc.scalar.copy(hT[:, ck, m * 128:(m + 1) * 128], pstv)

    # ---- MLP ----
    hid = mlp.tile([128, NGT, 512], BF16, name="hid")
    psacc = pool("psacc", 4, space="PSUM")
    ostg = pool("ostg", 3)
    for th in range(2):
        for gtg in range(NGT // 4):
            wfcg = wstream.tile([128, CK, 512], BF16, name="wfcg")
            nc.scalar.dma_start(out=wfcg, in_=d["wfc"][:, gtg])
            for gi in range(4):
                gt = gtg * 4 + gi
                psf = ps.tile([128, 512], F32, name="ps")
                for kt in range(CK):
                    nc.tensor.matmul(psf, wfcg[:, kt, gi * 128:(gi + 1) * 128],
                                     hT[:, kt, th * 512:(th + 1) * 512],
                                     start=(kt == 0), stop=(kt == CK - 1))
                nc.scalar.activation(hid[:, gt, :], psf, AF.Gelu,
                                     bias=bfc_sb[:, gt:gt + 1])
        for n in range(2):
            accs = [psacc.tile([128, 512], F32, name="acc") for _ in range(4)]
            for gkb in range(NGT // 4):
                wf2 = wstream.tile([128, 4, 512], BF16, name="wf2")
                nc.sync.dma_start(out=wf2, in_=d["wfc2"][:, n, gkb])
                for gi in range(4):
                    gkt = gkb * 4 + gi
                    for ml_ in range(4):
                        nc.tensor.matmul(accs[ml_],
                                         hid[:, gkt, ml_ * 128:(ml_ + 1) * 128],
                                         wf2[:, gi, :], start=(gkt == 0), stop=False)
            for ml_ in range(4):
                m = th * 4 + ml_
                nc.tensor.matmul(accs[ml_], ones_row,
                                 bfc2_sb[:, n * 512:(n + 1) * 512],
                                 start=False, stop=True)
                osb = ostg.tile([128, 512], F32, name="osb")
                nc.vector.tensor_tensor(osb, accs[ml_],
                                        x2[:, m, n * 512:(n + 1) * 512], op=OP.add)
                nc.sync.dma_start(out=out[m * 128:(m + 1) * 128,
                                          n * 512:(n + 1) * 512],
                                  in_=osb)


def make_masks():
    """[128, 4, 2, 512] bf16 diagonal masks, duplicated across the head dim."""
    m = np.zeros((4, 128, 512), np.float32)
    kk = np.arange(128)[:, None]
    qq = np.arange(512)[None, :]
    for j in range(4):
        m[j] = ((j * 128 + kk) <= qq).astype(np.float32)
    m2 = np.repeat(m[:, :, None, :], 2, axis=2)          # [4, 128, 2, 512]
    return np.ascontiguousarray(m2.transpose(1, 0, 2, 3)).astype(BF)


def make_in_maps(inputs):
    f32 = lambda a: np.asarray(a, dtype=np.float32)
    x = f32(inputs["x"])
    W_attn, b_attn = f32(inputs["W_attn"]), f32(inputs["b_attn"])
    W_proj, b_proj = f32(inputs["W_proj"]), f32(inputs["b_proj"])
    W_fc, b_fc = f32(inputs["W_fc"]), f32(inputs["b_fc"])
    W_fc2, b_fc2 = f32(inputs["W_fc2"]), f32(inputs["b_fc2"])
    g1, b1 = f32(inputs["ln1_g"]), f32(inputs["ln1_b"])
    g2, b2 = f32(inputs["ln2_g"]), f32(inputs["ln2_b"])

    # host-side constant folding (weight-only transforms)
    Wq = W_attn * g1[:, None]                        # [C, 3C]
    beta = b1 @ Wq + b_attn                          # [3C]
    Wf = W_fc * g2[:, None]                          # [C, 4C]
    beta2 = b2 @ Wf + b_fc                           # [4C]

    masks = make_masks()
    ebias = np.zeros((2, 128, 24), np.float32)
    ebias[0, :, 4:8] = NEG          # fold0: slot0 kt4..7 invisible
    ebias[1, :, 16:20] = NEG        # fold1: slot1 kt8..11 invisible

    ctg = np.ascontiguousarray
    wq_r = Wq.reshape(CK, 128, 3 * C).transpose(1, 0, 2)      # [p, k, 3C]
    shared = {
        "wvw": ctg(wq_r[:, :, 2 * C:].reshape(128, CK, 2, 512)
                   .transpose(0, 2, 1, 3)).astype(BF),
        "wkqw": ctg(wq_r[:, :, :2 * C].reshape(128, CK, 2, G, DG)
                    .transpose(0, 3, 1, 2, 4)).astype(BF),
        "wproj": ctg(W_proj.reshape(CK, 128, C).transpose(1, 0, 2)).astype(BF),
        "wfc": ctg(Wf.reshape(CK, 128, FC).transpose(1, 0, 2)
                   .reshape(128, CK, 8, 512).transpose(0, 2, 1, 3)).astype(BF),
        "wfc2": ctg(W_fc2.reshape(8, 4, 128, 2, 512)
                    .transpose(2, 3, 0, 1, 4)).astype(BF),
        "masks": masks,
        "beta_col": ctg(beta[:2 * C].reshape(16, 128).T).astype(np.float32),
        "betav_bc": ctg(np.broadcast_to(beta[2 * C:], (128, C))).astype(np.float32),
        "bfc_col": ctg(beta2.reshape(NGT, 128).T).astype(np.float32),
        "bfc2_row": b_fc2[None, :].astype(BF),
    }
    in_maps = []
    chunk_orders = ([0, 1, 2, 3], [1, 0, 3, 2])
    for c in range(8):
        b, f = c // 2, c % 2
        order = chunk_orders[f]
        xp = np.concatenate([x[b, 512 * ci:512 * (ci + 1)] for ci in order])
        xr = np.concatenate([xp[0:512], xp[1536:2048]]) + b_proj[None, :]
        in_maps.append(dict(
            shared,
            xT=ctg(xp.reshape(4, 512, CK, 128).transpose(3, 0, 2, 1)).astype(BF),
            xn=ctg(xp.reshape(4, 4, 128, C).transpose(2, 0, 1, 3)).astype(BF),
            xr=ctg(xr.reshape(NQT, 128, C).transpose(1, 0, 2)).astype(np.float32),
            ebias=ebias[f],
        ))
    return in_maps


def assemble_out(results):
    out = np.empty((B, T, C), np.float32)
    for c in range(8):
        b, f = c // 2, c % 2
        r = results[c]["out"]
        if f == 0:
            out[b, 0:512] = r[0:512]
            out[b, 1536:2048] = r[512:1024]
        else:
            out[b, 512:1024] = r[0:512]
            out[b, 1024:1536] = r[512:1024]
    return out


_NC_CACHE = {}


def kernel(**inputs):
    if "nc" not in _NC_CACHE:
        _NC_CACHE["nc"] = build_nc()
    nc = _NC_CACHE["nc"]
    in_maps = make_in_maps(inputs)
    rr = run_bass_kernel_spmd(nc, in_maps, list(range(8)))
    return assemble_out(rr.results)
